# revision 6
# baseline (speedup 1.0000x reference)
"""Gaussian upsampling embedding kernel for Trainium2 (8 NeuronCores).

Data-parallel over the batch dim: 32 batches -> 4 per core.

Math (per batch b):
  c_i   = cumsum(durs)_i - durs_i/2          (gaussian centers)
  sig_i = durs_i/2 + 1e-6
  w[t,i] = 1/(sig_i*sqrt(2pi)) * exp(-((t+0.5-c_i)/sig_i)^2/2)
  out[t,:] = sum_i w[t,i]*embed[text_i] / sum_i w[t,i]          (t < total_dur)
  out[t,:] = embed[0]                                           (t >= total_dur)

The t >= total_dur rows are filled on the host (exact), so the device only
computes chunks below each batch's padded duration; rows between total_dur
and the chunk boundary may hold garbage (host overwrites them).

Device pipeline per batch (engines overlap under Tile):
  ACT : g[i,t] = Derivative_Erf(s_i*tval[t] + b_i)  (= 2/sqrt(pi)*exp(-z^2)),
        fp16 output, restricted to the t-span where some char of the half
        has |z'| < MARGIN (outside, w underflows to 0 = reference behavior)
  PE  : O[t,:] = sum over char halves q of g_q[:,tchunk]^T @ Eg_q   (fp16 mm)
        Eg_q[i,:] = amp_i * embed[text_i] with an extra amp_i column
        (-> O[:,384] = row-sum S); halves whose span misses the chunk skip.
  DVE : recip[t] = 1/S (two 128-row chunks per op via strided PSUM AP)
  ACT/DVE/Pool : out = O[:,:384]*recip, fp32 PSUM -> fp16 SBUF, chunks
        round-robined across the three engines to balance busy time.

All off-chip traffic is fp16 (tolerance 2e-2 >> fp16 rounding): gathered
embeddings in, final output out. Input DMAs are issued from different
engines in parallel (each dma_start costs ~650ns of issuing-engine time).
"""

import os
import numpy as np
from contextlib import ExitStack

_B, _T, _V, _D = 32, 256, 100, 384
_NC = 8
_BPC = _B // _NC    # batches per core
_EPS = np.float32(1e-6)
_MARGIN = 7.5       # |z'| beyond which exp(-z'^2) flushes to 0 in fp16
_W = _D + 2         # matmul N: 384 emb cols + amp col (S) + 1 pad

# Set by kernel() after each run (for the local test harness).
LAST_RESULT = None


def _build_program(Tt, spans, nt_b):
    """spans[b][q] = (c_lo, c_hi) 128-chunk index range char half q of
    batch-slot b contributes to (union across cores). nt_b[b] = number of
    128-chunks this slot computes/stores (union across cores)."""
    import concourse.bass as bass
    import concourse.tile as tile
    from concourse import bacc, mybir

    f32 = mybir.dt.float32
    f16 = mybir.dt.float16
    AF = mybir.ActivationFunctionType

    NT = (Tt + 127) // 128          # output t-chunks of 128 rows
    NTP = NT * 128

    nc = bacc.Bacc(
        "TRN2",
        target_bir_lowering=False,
        debug=False,
        num_devices=_NC,
    )

    coef = nc.dram_tensor("coef", [128, _BPC * 2 * 2], f32, kind="ExternalInput").ap()
    egp = nc.dram_tensor("egp", [_BPC, 128, 2 * _W], f16, kind="ExternalInput").ap()
    out = nc.dram_tensor("out", [_BPC, NTP, _D], f16, kind="ExternalOutput").ap()

    with tile.TileContext(nc) as tc, ExitStack() as ctx:
        const = ctx.enter_context(tc.tile_pool(name="const", bufs=1))
        wpool = ctx.enter_context(tc.tile_pool(name="wT", bufs=8))
        opool = ctx.enter_context(tc.tile_pool(name="osb", bufs=6))
        rpool = ctx.enter_context(tc.tile_pool(name="recip", bufs=10))
        pso = ctx.enter_context(tc.tile_pool(name="pso", bufs=2, space="PSUM"))

        # input DMAs: only Sync/ACT/Pool can initiate DMAs. Sync moves coef
        # + the first two batches (earliest consumers); Pool (software DGE)
        # moves the last two after the iota. ACT goes straight to gaussians.
        coef_sb = const.tile([128, _BPC * 2 * 2], f32)
        nc.sync.dma_start(coef_sb[:], coef[:])
        eg_sb = const.tile([128, _BPC * 2 * _W], f16)
        dma_eng = [nc.sync, nc.sync, nc.gpsimd, nc.gpsimd]

        def load_eg(bb):
            w0 = bb * 2 * _W
            dma_eng[bb].dma_start(eg_sb[:, w0 : w0 + 2 * _W], egp[bb])

        load_eg(0)
        load_eg(1)
        # tval = arange(NTP) replicated on all partitions (f32 iota is exact
        # below 2^24); the +0.5 frame offset is folded into b_coef on host.
        # Split at batch 0's first-half span end so its gaussian eval can
        # start before the full ramp is generated.
        tval_sb = const.tile([128, NTP], f32)
        sp = min(spans[0][0][1] * 128, NTP)
        nc.gpsimd.iota(
            tval_sb[:, :sp], [[1, sp]], channel_multiplier=0,
            allow_small_or_imprecise_dtypes=True,
        )
        if sp < NTP:
            nc.gpsimd.iota(
                tval_sb[:, sp:], [[1, NTP - sp]], base=sp, channel_multiplier=0,
                allow_small_or_imprecise_dtypes=True,
            )
        load_eg(2)
        load_eg(3)

        def cf(b, q, c):
            j = (b * 2 + q) * 2 + c
            return coef_sb[:, j : j + 1]

        def eg(b, q):
            j = (b * 2 + q) * _W
            return eg_sb[:, j : j + _W]

        # normalize alternates ACT/DVE per chunk (GPSIMD cannot read PSUM);
        # DVE also does the reciprocals and ACT the gaussian evals
        nrr = 0
        G = 4  # t-chunks per PSUM tile / store group

        for b in range(_BPC):
            NTb = nt_b[b]
            # Gaussian eval restricted to contributing span (fp16 out)
            wT = []
            for q in range(2):
                lo, hi = spans[b][q][0] * 128, spans[b][q][1] * 128
                w = wpool.tile([128, NTP], f16, tag="wT")
                nc.scalar.activation(
                    w[:, lo:hi],
                    tval_sb[:, lo:hi],
                    AF.Derivative_Erf,
                    scale=cf(b, q, 0),
                    bias=cf(b, q, 1),
                )
                wT.append(w)

            # out chunks: O[t, 0:384] unnormalized embedding, O[t, 384] = S
            for g in range((NTb + G - 1) // G):
                ilist = [i for i in range(G * g, G * g + G) if i < NTb]
                ng = len(ilist)
                po = pso.tile([128, 512 * ng], f32, tag="pso")
                for j, i in enumerate(ilist):
                    dst = po[:, j * 512 : j * 512 + _W]
                    qs = [
                        q
                        for q in range(2)
                        if spans[b][q][0] <= i < spans[b][q][1]
                    ]
                    assert qs, f"t-chunk {i} of slot {b} has no contribution"
                    for k, q in enumerate(qs):
                        nc.tensor.matmul(
                            dst,
                            wT[q][:, i * 128 : (i + 1) * 128],
                            eg(b, q),
                            start=(k == 0),
                            stop=(k == len(qs) - 1),
                        )
                rc = rpool.tile([128, G], f32, tag="recip")
                nc.vector.reciprocal(
                    rc[:, :ng], po[:, _D : _D + 512 * (ng - 1) + 1 : 512]
                )
                out_sb = opool.tile([128, ng * _D], f16, tag="osb")
                for j, i in enumerate(ilist):
                    dst = out_sb[:, j * _D : (j + 1) * _D]
                    src = po[:, j * 512 : j * 512 + _D]
                    if nrr % 2 == 0:
                        nc.scalar.activation(
                            dst, src, AF.Copy, scale=rc[:, j : j + 1]
                        )
                    else:
                        nc.vector.tensor_scalar_mul(dst, src, rc[:, j : j + 1])
                    nrr += 1
                # flush this group (host discards rows >= Tt and overwrites
                # rows >= per-batch total_dur)
                nc.sync.dma_start(
                    out[b, G * g * 128 : (G * g + ng) * 128].rearrange(
                        "(i p) d -> p i d", p=128
                    ),
                    out_sb[:].rearrange("p (i d) -> p i d", d=_D),
                )

    nc.compile()
    return nc


def _host_prep(text, durs, embed, Tt):
    """Per-core input maps + per-slot contribution spans on the 128-grid."""
    text_i = np.asarray(text).astype(np.int64)          # [32, 256]
    durs_f = np.asarray(durs).astype(np.float32)        # [32, 256]
    embed = np.asarray(embed, dtype=np.float32)         # [100, 384]

    NT = (Tt + 127) // 128
    NTP = NT * 128

    csum = np.cumsum(durs_f, axis=-1, dtype=np.float32)
    c = csum - durs_f / 2.0                             # centers
    sig = durs_f / 2.0 + _EPS
    sq2 = np.float32(np.sqrt(2.0))
    s_coef = (1.0 / (sig * sq2)).astype(np.float32)
    b_coef = ((0.5 - c) / (sig * sq2)).astype(np.float32)
    amp = (1.0 / (2.0 * sq2 * sig)).astype(np.float32)
    td = np.asarray(durs).astype(np.int64).sum(axis=-1)  # [32]

    # chunks computed per slot: enough to cover every core's total_dur
    nt_slot = np.minimum(-(-td // 128), NT).reshape(_NC, _BPC).max(axis=0)
    nt_b = tuple(int(x) for x in nt_slot)

    # contribution spans per (batch, char-half) on the 128-chunk grid,
    # unioned across the 8 cores (SPMD-shared program), clipped to nt_b
    lo_t = np.clip(c - _MARGIN * sig * sq2, 0, NTP).reshape(_B, 2, 128).min(axis=2)
    hi_t = np.clip(c + _MARGIN * sig * sq2 + 1, 0, NTP).reshape(_B, 2, 128).max(axis=2)
    lo_s = lo_t.reshape(_NC, _BPC, 2).min(axis=0)        # [BPC, 2]
    hi_s = hi_t.reshape(_NC, _BPC, 2).max(axis=0)
    spans = []
    for b in range(_BPC):
        row = []
        for q in range(2):
            c_lo = max(0, min(int(lo_s[b, q]) // 128, nt_b[b] - 1))
            c_hi = max(c_lo + 1, min(-(-int(hi_s[b, q]) // 128), nt_b[b]))
            row.append((c_lo, c_hi))
        spans.append(tuple(row))
    spans = tuple(spans)

    # coef layout: [128 partitions, (b, q, c)] with c = (s, b)
    stack = np.stack([s_coef, b_coef], axis=-1)          # [32, 256, 2]
    stack = stack.reshape(_B, 2, 128, 2)                 # [32, q, p, c]

    # gathered, amplitude-folded embeddings + amp column (row-sum), fp16,
    # pre-transposed to the SBUF layout [128 chars, (q, d)]
    egp = np.zeros((_B, 2, 128, _W), np.float16)
    gat = embed[text_i]                                  # [32, 256, 384]
    egp[:, :, :, :_D] = (gat * amp[:, :, None]).reshape(_B, 2, 128, _D)
    egp[:, :, :, _D] = amp.reshape(_B, 2, 128)
    egp = egp.transpose(0, 2, 1, 3).reshape(_B, 128, 2 * _W)

    in_maps = []
    for core in range(_NC):
        bs = slice(core * _BPC, (core + 1) * _BPC)
        coef_core = (
            stack[bs].transpose(2, 0, 1, 3).reshape(128, _BPC * 2 * 2).copy()
        )
        in_maps.append(
            {
                "coef": coef_core,
                "egp": egp[bs].copy(),
            }
        )
    return in_maps, spans, nt_b, td


def kernel(text, durs, embed, total_time):
    global LAST_RESULT
    from concourse.bass_utils import run_bass_kernel_spmd

    Tt = int(total_time)
    in_maps, spans, nt_b, td = _host_prep(text, durs, embed, Tt)
    nc = _build_program(Tt, spans, nt_b)

    trace = bool(int(os.environ.get("GK_TRACE", "0")))
    res = run_bass_kernel_spmd(
        nc, in_maps, list(range(_NC)), trace=trace
    )
    LAST_RESULT = res
    out = np.concatenate([r["out"][:, :Tt] for r in res.results], axis=0)
    out = out.astype(np.float32)
    # rows at/past each utterance's total duration are exactly embed[0]
    emb0 = np.asarray(embed, dtype=np.float32)[0]
    tgrid = np.arange(Tt)[None, :]
    pad = tgrid >= np.asarray(td)[:, None]               # [32, Tt]
    out[pad] = emb0
    return np.ascontiguousarray(out)


if __name__ == "__main__":
    rng = np.random.default_rng(0)
    text = rng.integers(1, _V, size=(_B, _T), dtype=np.int64)
    durs = rng.integers(1, 9, size=(_B, _T), dtype=np.int32)
    embed = rng.normal(size=(_V, _D)).astype(np.float32)
    Tt = int(durs.sum(axis=-1).max())
    o = kernel(text, durs, embed, Tt)
    print("out", o.shape, o.dtype)
